# revision 45
# baseline (speedup 1.0000x reference)
"""CommutatorConv2d kernel for Trainium2 (Bass/Tile), 8-core data-parallel.

Math: the reference's commutator/anticommutator conv reduces exactly to a
single-channel 3x3 conv on the channel-summed input:

    out[b] = T @ xs[b] @ A + Bm @ xs[b] @ T + bias,   xs = x.sum(axis=1)

where T is the 128x128 tridiagonal-ones matrix and A, Bm are tridiagonal
matrices built from K's column/row sums scaled by (lambda_c +/- lambda_a):
sum_{i,m} XK[...,i,m] = sum_{i,j} patch[i,j]*colsum(K)[j] and
sum_{j,i} KX[...,j,i] = sum_{m,i} patch[m,i]*rowsum(K)[m], so the effective
3x3 kernel is W[i,j] = a[j] + b[i], separable into a row-conv on the vertical
boxsum plus a col-conv on the horizontal boxsum = the two matrix sandwiches.

Precision: x and the constant matrices are cast to bf16 on the host. All
accumulations happen in fp32 PSUM; only element roundings are bf16, giving
~4e-3 relative error against the fp32 reference (the gate is 2e-2) while
halving HBM traffic (the kernel is HBM-bound) and doubling PE/DVE rates.

Layout: each core's shard is sent as [H, B_loc, C, W] (h-major) so every
SBUF partition receives one contiguous >=512B DRAM run per piece.

Schedule: every load is issued up-front, alternating between the two HWDGE
queues, so the HBM path streams without waiting on tile or semaphore
recycling (the previous revision's 8.6us of queue-idle gaps). Per batch,
pieces 0-1 fold on the tensor engine as bf16 identity-matmul quads into one
fp32 PSUM accumulator; pieces 2-3 fold on the vector engine as in-place
binary trees. The partial sums are never merged into one xs: uv accumulates
all three (quad partial + two tree results) with back-to-back matmuls
against [T | Bm.T]. Batch 0's uv consumes the four evac column groups
directly (one dependency hop after the ACT evac, so its whole sandwich
schedules ahead of batch 1's quads and the two chains never collide on
PE/ACT); batch 1 folds the groups on DVE first so the fewest matmuls
remain after the final piece lands on its tail-critical path. The
activation engine does every PSUM evacuation, stage 2 multiplies the uv
halves by A and T, bias adds straight out of PSUM in fp32, and both
stores ride the sync queue (a store issue on the scalar queue would block
the other batch's ACT ops behind it). Engine assignments were A/B'd on hardware; notable dead ends:
GpSimd folds (DVE's 2-port bf16 mode locks GpSimd out of SBUF, +6us), fp32r
matmuls (all-zero output on TRN2), single-queue streaming (~250 GB/s vs
~310 combined), and batch-role-aware queue layouts (cross-queue landing
jitter flips their intended orderings run to run).
"""

import numpy as np

B, C, H, W = 16, 32, 128, 128
N_CORES = 8
B_LOC = B // N_CORES

_PROGRAM = None
LAST_RESULTS = None


def _build_program():
    import concourse.mybir as mybir
    from concourse import bacc
    from concourse.bass import MemorySpace
    from concourse.tile import TileContext

    f32 = mybir.dt.float32
    bf16 = mybir.dt.bfloat16
    nc = bacc.Bacc(
        "TRN2", target_bir_lowering=False, debug=False, num_devices=N_CORES
    )

    x_dram = nc.dram_tensor("x", (H, B_LOC, C, W), bf16, kind="ExternalInput")
    # fused constants: [T | Bm.T | A | I | bias] as bf16 columns; the last
    # two columns hold each partition's fp32 bias value as raw bits
    cm_dram = nc.dram_tensor("cmat", (H, 4 * W + 2), bf16, kind="ExternalInput")
    out_dram = nc.dram_tensor("out", (H, B_LOC, W), f32, kind="ExternalOutput")

    x_ap = x_dram.ap()
    out_ap = out_dram.ap()

    PIECE = 8  # channels per full piece

    with TileContext(nc) as tc:
        with (
            tc.tile_pool(name="consts", bufs=1) as cpool,
            tc.tile_pool(name="xpool", bufs=2) as xpool,
            tc.tile_pool(name="spool", bufs=2) as spool,
            tc.tile_pool(name="psum", bufs=2, space=MemorySpace.PSUM) as ppool,
        ):
            cm_sb = cpool.tile([H, 4 * W + 2], bf16)
            t_sb = cm_sb[:, 0:W]
            tbm_sb = cm_sb[:, 0 : 2 * W]
            a_sb = cm_sb[:, 2 * W : 3 * W]
            i_sb = cm_sb[:, 3 * W : 4 * W]
            bias_sb = cm_sb[:, 4 * W : 4 * W + 2].bitcast(f32)

            # ---- phase 0: every load issued up-front. Pieces 0/1 are PE
            # quad pieces, pieces 2/3 DVE tree pieces; 8 channels each.
            tiles = {}
            for b in range(B_LOC):
                for p in range(4):
                    tiles[(b, p)] = xpool.tile(
                        [H, PIECE * W], bf16, tag=f"xq{p}", name=f"xq{b}{p}"
                    )

            def load(eng, b, p, c0, c1, col0):
                xq = tiles[(b, p)]
                eng.dma_start(
                    out=xq[:, col0 * W : (col0 + (c1 - c0)) * W].rearrange(
                        "h (c w) -> h c w", w=W
                    ),
                    in_=x_ap[:, b, c0:c1, :],
                )

            nc.scalar.dma_start(out=cm_sb, in_=cm_dram.ap())
            # Batch 0's quad pieces take the two earliest sync positions so
            # its evac chain is early under ANY landing model (the scheduler
            # otherwise coalesces b0's evac behind b1's quads); batch 0's
            # tree pieces land last, where its evac-direct uv gives the
            # shortest post-landing chain, while batch 1 (csf path) gets the
            # middle of the stream.
            for b, p in [(0, 0), (0, 1), (1, 2), (1, 3)]:
                load(nc.sync, b, p, p * PIECE, (p + 1) * PIECE, 0)
            for b, p in [(1, 0), (1, 1), (0, 2), (0, 3)]:
                load(nc.scalar, b, p, p * PIECE, (p + 1) * PIECE, 0)

            # ---- phase 1, per batch
            for b in range(B_LOC):
                # PE: 4 identity quads fold pieces 0-1 into fp32 PSUM
                quad_pieces = (0, 1)
                nquads = 2 * len(quad_pieces)
                cs_psum = ppool.tile([H, 4 * W], f32, tag="csp")
                q = 0
                for p in quad_pieces:
                    for half in range(2):
                        nc.tensor.matmul(
                            cs_psum,
                            i_sb,
                            tiles[(b, p)][:, half * 4 * W : (half + 1) * 4 * W],
                            start=(q == 0),
                            stop=(q == nquads - 1),
                        )
                        q += 1
                # ACT evacuates the 4-way partial (rounding to bf16)
                cs = spool.tile([H, 4 * W], bf16, tag="cs")
                nc.scalar.copy(cs, cs_psum)

                # DVE folds the quad partial (batch 1 only; batch 0's uv
                # consumes the evac groups directly)
                if b != 0:
                    nc.vector.tensor_add(
                        cs[:, 0 : 2 * W], cs[:, 0 : 2 * W], cs[:, 2 * W : 4 * W]
                    )
                    nc.vector.tensor_add(cs[:, 0:W], cs[:, 0:W], cs[:, W : 2 * W])
                tree_pieces = (2, 3)
                for p in tree_pieces:
                    xq = tiles[(b, p)]
                    n = PIECE * W
                    while n > W:
                        n //= 2
                        nc.vector.tensor_add(xq[:, :n], xq[:, :n], xq[:, n : 2 * n])

                # PE: uv accumulates every partial sum against [T | Bm.T]
                uv_psum = ppool.tile([H, 2 * W], f32, tag="uvp")
                cs_parts = (
                    [cs[:, g * W : (g + 1) * W] for g in range(4)]
                    if b == 0
                    else [cs[:, 0:W]]
                )
                for i, part in enumerate(cs_parts):
                    nc.tensor.matmul(
                        uv_psum, part, tbm_sb, start=(i == 0), stop=False
                    )
                for p in tree_pieces:
                    nc.tensor.matmul(
                        uv_psum,
                        tiles[(b, p)][:, 0:W],
                        tbm_sb,
                        start=False,
                        stop=(p == 3),
                    )
                uv = spool.tile([H, 2 * W], bf16, tag="uv")
                # split copies so stage-2's first matmul starts after half
                nc.scalar.copy(uv[:, 0:W], uv_psum[:, 0:W])
                nc.scalar.copy(uv[:, W : 2 * W], uv_psum[:, W : 2 * W])

                op = ppool.tile([H, W], f32, tag="op")
                nc.tensor.matmul(op, uv[:, 0:W], a_sb, start=True, stop=False)
                nc.tensor.matmul(op, uv[:, W : 2 * W], t_sb, start=False, stop=True)

                o2b = spool.tile([H, W], f32, tag="o2")
                nc.scalar.add(o2b, op, add=bias_sb)
                nc.sync.dma_start(out=out_ap[:, b, :], in_=o2b)

    nc.compile()
    return nc


def _get_program():
    global _PROGRAM
    if _PROGRAM is None:
        _PROGRAM = _build_program()
    return _PROGRAM


def _build_consts(K, bias, lambda_c, lambda_a, np_bf16):
    K = np.asarray(K, np.float32)
    lc = float(np.asarray(lambda_c))
    la = float(np.asarray(lambda_a))
    a = (lc + la) * K.sum(axis=0)  # column sums -> horizontal taps
    b = (la - lc) * K.sum(axis=1)  # row sums -> vertical taps
    eye = np.eye(H, dtype=np.float32)
    up = np.eye(H, k=1, dtype=np.float32)
    dn = np.eye(H, k=-1, dtype=np.float32)
    T = eye + up + dn
    A = a[1] * eye + a[0] * up + a[2] * dn
    Bm = b[1] * eye + b[2] * up + b[0] * dn
    # fused [T | Bm.T | A | I] in bf16, then the fp32 bias bit-packed into
    # two trailing bf16 columns
    cm = np.concatenate([T, Bm.T, A, eye], axis=1).astype(np_bf16)
    bias_col = np.full(
        (H, 1), np.asarray(bias, np.float32).reshape(-1)[0], np.float32
    )
    bias_bits = bias_col.view(np.uint16).view(np_bf16)  # [H, 2] raw halves
    return np.ascontiguousarray(np.concatenate([cm, bias_bits], axis=1))


def kernel(x, K, bias, lambda_c, lambda_a, _trace=False):
    global LAST_RESULTS
    import concourse.mybir as mybir
    from concourse.bass_utils import run_bass_kernel_spmd

    np_bf16 = mybir.dt.np(mybir.dt.bfloat16)
    x = np.asarray(x, np.float32)
    cm = _build_consts(K, bias, lambda_c, lambda_a, np_bf16)
    nc = _get_program()

    in_maps = []
    for core in range(N_CORES):
        shard = x[core * B_LOC : (core + 1) * B_LOC]  # [B_LOC, C, H, W]
        shard_t = np.ascontiguousarray(
            shard.transpose(2, 0, 1, 3).astype(np_bf16)
        )  # [H, B, C, W] bf16
        in_maps.append({"x": shard_t, "cmat": cm})

    res = run_bass_kernel_spmd(
        nc, in_maps, core_ids=list(range(N_CORES)), trace=_trace
    )
    LAST_RESULTS = res
    # per-core outputs are [H, B_LOC, W]; swap back to [B_LOC, H, W]
    out = np.concatenate(
        [r["out"].transpose(1, 0, 2) for r in res.results], axis=0
    )
    return out.reshape(B, 1, H, W).astype(np.float32, copy=False)


# revision 47
# speedup vs baseline: 1.0866x; 1.0866x over previous
"""CommutatorConv2d kernel for Trainium2 (Bass/Tile), 8-core data-parallel.

Math: the reference's commutator/anticommutator conv reduces exactly to a
single-channel 3x3 conv on the channel-summed input:

    out[b] = T @ xs[b] @ A + Bm @ xs[b] @ T + bias,   xs = x.sum(axis=1)

where T is the 128x128 tridiagonal-ones matrix and A, Bm are tridiagonal
matrices built from K's column/row sums scaled by (lambda_c +/- lambda_a):
sum_{i,m} XK[...,i,m] = sum_{i,j} patch[i,j]*colsum(K)[j] and
sum_{j,i} KX[...,j,i] = sum_{m,i} patch[m,i]*rowsum(K)[m], so the effective
3x3 kernel is W[i,j] = a[j] + b[i], separable into a row-conv on the vertical
boxsum plus a col-conv on the horizontal boxsum = the two matrix sandwiches.

Precision: x and the constant matrices are cast to bf16 on the host. All
accumulations happen in fp32 PSUM; only element roundings are bf16, giving
~4e-3 relative error against the fp32 reference (the gate is 2e-2) while
halving HBM traffic (the kernel is HBM-bound) and doubling PE/DVE rates.

Layout: each core's shard is sent as [H, B_loc, C, W] (h-major) so every
SBUF partition receives one contiguous >=512B DRAM run per piece.

Schedule: every load is issued up-front, alternating between the two HWDGE
queues, so the HBM path streams without waiting on tile or semaphore
recycling (the previous revision's 8.6us of queue-idle gaps). Per batch,
pieces 0-1 fold on the tensor engine as bf16 identity-matmul quads into one
fp32 PSUM accumulator; pieces 2-3 fold on the vector engine as in-place
binary trees. The partial sums are never merged into one xs: uv accumulates
all three (quad partial + two tree results) with back-to-back matmuls
against [T | Bm.T]. Batch 0's uv consumes the four evac column groups
directly (one dependency hop after the ACT evac, so its whole sandwich
schedules ahead of batch 1's quads and the two chains never collide on
PE/ACT); batch 1 folds the groups on DVE first so the fewest matmuls
remain after the final piece lands on its tail-critical path. The
activation engine does every PSUM evacuation, stage 2 multiplies the uv
halves by A and T, bias adds straight out of PSUM in fp32, and both
stores ride the sync queue (a store issue on the scalar queue would block
the other batch's ACT ops behind it). Engine assignments were A/B'd on hardware; notable dead ends:
GpSimd folds (DVE's 2-port bf16 mode locks GpSimd out of SBUF, +6us), fp32r
matmuls (all-zero output on TRN2), single-queue streaming (~250 GB/s vs
~310 combined), and batch-role-aware queue layouts (cross-queue landing
jitter flips their intended orderings run to run).
"""

import numpy as np

B, C, H, W = 16, 32, 128, 128
N_CORES = 8
B_LOC = B // N_CORES

_PROGRAM = None
LAST_RESULTS = None


def _build_program():
    import concourse.mybir as mybir
    from concourse import bacc
    from concourse.bass import MemorySpace
    from concourse.tile import TileContext

    f32 = mybir.dt.float32
    bf16 = mybir.dt.bfloat16
    nc = bacc.Bacc(
        "TRN2", target_bir_lowering=False, debug=False, num_devices=N_CORES
    )

    x_dram = nc.dram_tensor("x", (H, B_LOC, C, W), bf16, kind="ExternalInput")
    # fused constants: [T | Bm.T | A | I | bias] as bf16 columns; the last
    # two columns hold each partition's fp32 bias value as raw bits
    cm_dram = nc.dram_tensor("cmat", (H, 4 * W + 2), bf16, kind="ExternalInput")
    out_dram = nc.dram_tensor("out", (H, B_LOC, W), f32, kind="ExternalOutput")

    x_ap = x_dram.ap()
    out_ap = out_dram.ap()

    PIECE = 8  # channels per full piece

    with TileContext(nc) as tc:
        with (
            tc.tile_pool(name="consts", bufs=1) as cpool,
            tc.tile_pool(name="xpool", bufs=2) as xpool,
            tc.tile_pool(name="spool", bufs=2) as spool,
            tc.tile_pool(name="psum", bufs=2, space=MemorySpace.PSUM) as ppool,
        ):
            cm_sb = cpool.tile([H, 4 * W + 2], bf16)
            t_sb = cm_sb[:, 0:W]
            tbm_sb = cm_sb[:, 0 : 2 * W]
            a_sb = cm_sb[:, 2 * W : 3 * W]
            i_sb = cm_sb[:, 3 * W : 4 * W]
            bias_sb = cm_sb[:, 4 * W : 4 * W + 2].bitcast(f32)

            # ---- phase 0: every load issued up-front. Pieces 0/1 are PE
            # quad pieces, pieces 2/3 DVE tree pieces; 8 channels each.
            tiles = {}
            for b in range(B_LOC):
                for p in range(4):
                    tiles[(b, p)] = xpool.tile(
                        [H, PIECE * W], bf16, tag=f"xq{p}", name=f"xq{b}{p}"
                    )

            def load(eng, b, p, c0, c1, col0):
                xq = tiles[(b, p)]
                eng.dma_start(
                    out=xq[:, col0 * W : (col0 + (c1 - c0)) * W].rearrange(
                        "h (c w) -> h c w", w=W
                    ),
                    in_=x_ap[:, b, c0:c1, :],
                )

            nc.scalar.dma_start(out=cm_sb, in_=cm_dram.ap())
            order = [(0, 0), (0, 1), (0, 2), (0, 3), (1, 0), (1, 1), (1, 2)]
            for idx, (b, p) in enumerate(order):
                eng = nc.sync if idx % 2 == 0 else nc.scalar
                load(eng, b, p, p * PIECE, (p + 1) * PIECE, 0)
            # batch 1's final piece arrives as a 6ch run plus a 2ch sliver
            # landing last, so the tail fold is one add + one merge instead
            # of a 3-op tree
            load(nc.scalar, 1, 3, 24, 30, 0)
            load(nc.sync, 1, 3, 30, 32, 6)

            # ---- phase 1, per batch
            for b in range(B_LOC):
                # PE: 4 identity quads fold pieces 0-1 into fp32 PSUM
                quad_pieces = (0, 1)
                nquads = 2 * len(quad_pieces)
                cs_psum = ppool.tile([H, 4 * W], f32, tag="csp")
                q = 0
                for p in quad_pieces:
                    for half in range(2):
                        nc.tensor.matmul(
                            cs_psum,
                            i_sb,
                            tiles[(b, p)][:, half * 4 * W : (half + 1) * 4 * W],
                            start=(q == 0),
                            stop=(q == nquads - 1),
                        )
                        q += 1
                # ACT evacuates the 4-way partial (rounding to bf16)
                cs = spool.tile([H, 4 * W], bf16, tag="cs")
                nc.scalar.copy(cs, cs_psum)

                # DVE folds the quad partial (batch 1 only; batch 0's uv
                # consumes the evac groups directly)
                if b != 0:
                    nc.vector.tensor_add(
                        cs[:, 0 : 2 * W], cs[:, 0 : 2 * W], cs[:, 2 * W : 4 * W]
                    )
                    nc.vector.tensor_add(cs[:, 0:W], cs[:, 0:W], cs[:, W : 2 * W])
                tree_pieces = (2, 3)
                xq = tiles[(b, 2)]
                n = PIECE * W
                while n > W:
                    n //= 2
                    nc.vector.tensor_add(xq[:, :n], xq[:, :n], xq[:, n : 2 * n])
                p3 = tiles[(b, 3)]
                if b == 0:
                    n = PIECE * W
                    while n > W:
                        n //= 2
                        nc.vector.tensor_add(p3[:, :n], p3[:, :n], p3[:, n : 2 * n])
                else:
                    nc.vector.tensor_add(
                        p3[:, 0 : 2 * W], p3[:, 0 : 2 * W], p3[:, 2 * W : 4 * W]
                    )
                    nc.vector.tensor_add(
                        p3[:, 0 : 2 * W], p3[:, 0 : 2 * W], p3[:, 4 * W : 6 * W]
                    )
                    nc.vector.tensor_add(p3[:, 0:W], p3[:, 0:W], p3[:, W : 2 * W])
                    nc.vector.tensor_add(
                        p3[:, 6 * W : 7 * W],
                        p3[:, 6 * W : 7 * W],
                        p3[:, 7 * W : 8 * W],
                    )
                    nc.vector.tensor_add(p3[:, 0:W], p3[:, 0:W], p3[:, 6 * W : 7 * W])

                # PE: uv accumulates every partial sum against [T | Bm.T]
                uv_psum = ppool.tile([H, 2 * W], f32, tag="uvp")
                cs_parts = (
                    [cs[:, g * W : (g + 1) * W] for g in range(4)]
                    if b == 0
                    else [cs[:, 0:W]]
                )
                for i, part in enumerate(cs_parts):
                    nc.tensor.matmul(
                        uv_psum, part, tbm_sb, start=(i == 0), stop=False
                    )
                for p in tree_pieces:
                    nc.tensor.matmul(
                        uv_psum,
                        tiles[(b, p)][:, 0:W],
                        tbm_sb,
                        start=False,
                        stop=(p == 3),
                    )
                uv = spool.tile([H, 2 * W], bf16, tag="uv")
                # split copies so stage-2's first matmul starts after half
                nc.scalar.copy(uv[:, 0:W], uv_psum[:, 0:W])
                nc.scalar.copy(uv[:, W : 2 * W], uv_psum[:, W : 2 * W])

                op = ppool.tile([H, W], f32, tag="op")
                nc.tensor.matmul(op, uv[:, 0:W], a_sb, start=True, stop=False)
                nc.tensor.matmul(op, uv[:, W : 2 * W], t_sb, start=False, stop=True)

                o2b = spool.tile([H, W], f32, tag="o2")
                nc.scalar.add(o2b, op, add=bias_sb)
                nc.sync.dma_start(out=out_ap[:, b, :], in_=o2b)

    nc.compile()
    return nc


def _get_program():
    global _PROGRAM
    if _PROGRAM is None:
        _PROGRAM = _build_program()
    return _PROGRAM


def _build_consts(K, bias, lambda_c, lambda_a, np_bf16):
    K = np.asarray(K, np.float32)
    lc = float(np.asarray(lambda_c))
    la = float(np.asarray(lambda_a))
    a = (lc + la) * K.sum(axis=0)  # column sums -> horizontal taps
    b = (la - lc) * K.sum(axis=1)  # row sums -> vertical taps
    eye = np.eye(H, dtype=np.float32)
    up = np.eye(H, k=1, dtype=np.float32)
    dn = np.eye(H, k=-1, dtype=np.float32)
    T = eye + up + dn
    A = a[1] * eye + a[0] * up + a[2] * dn
    Bm = b[1] * eye + b[2] * up + b[0] * dn
    # fused [T | Bm.T | A | I] in bf16, then the fp32 bias bit-packed into
    # two trailing bf16 columns
    cm = np.concatenate([T, Bm.T, A, eye], axis=1).astype(np_bf16)
    bias_col = np.full(
        (H, 1), np.asarray(bias, np.float32).reshape(-1)[0], np.float32
    )
    bias_bits = bias_col.view(np.uint16).view(np_bf16)  # [H, 2] raw halves
    return np.ascontiguousarray(np.concatenate([cm, bias_bits], axis=1))


def kernel(x, K, bias, lambda_c, lambda_a, _trace=False):
    global LAST_RESULTS
    import concourse.mybir as mybir
    from concourse.bass_utils import run_bass_kernel_spmd

    np_bf16 = mybir.dt.np(mybir.dt.bfloat16)
    x = np.asarray(x, np.float32)
    cm = _build_consts(K, bias, lambda_c, lambda_a, np_bf16)
    nc = _get_program()

    in_maps = []
    for core in range(N_CORES):
        shard = x[core * B_LOC : (core + 1) * B_LOC]  # [B_LOC, C, H, W]
        shard_t = np.ascontiguousarray(
            shard.transpose(2, 0, 1, 3).astype(np_bf16)
        )  # [H, B, C, W] bf16
        in_maps.append({"x": shard_t, "cmat": cm})

    res = run_bass_kernel_spmd(
        nc, in_maps, core_ids=list(range(N_CORES)), trace=_trace
    )
    LAST_RESULTS = res
    # per-core outputs are [H, B_LOC, W]; swap back to [B_LOC, H, W]
    out = np.concatenate(
        [r["out"].transpose(1, 0, 2) for r in res.results], axis=0
    )
    return out.reshape(B, 1, H, W).astype(np.float32, copy=False)


# revision 48
# speedup vs baseline: 1.1082x; 1.0198x over previous
"""CommutatorConv2d kernel for Trainium2 (Bass/Tile), 8-core data-parallel.

Math: the reference's commutator/anticommutator conv reduces exactly to a
single-channel 3x3 conv on the channel-summed input:

    out[b] = T @ xs[b] @ A + Bm @ xs[b] @ T + bias,   xs = x.sum(axis=1)

where T is the 128x128 tridiagonal-ones matrix and A, Bm are tridiagonal
matrices built from K's column/row sums scaled by (lambda_c +/- lambda_a):
sum_{i,m} XK[...,i,m] = sum_{i,j} patch[i,j]*colsum(K)[j] and
sum_{j,i} KX[...,j,i] = sum_{m,i} patch[m,i]*rowsum(K)[m], so the effective
3x3 kernel is W[i,j] = a[j] + b[i], separable into a row-conv on the vertical
boxsum plus a col-conv on the horizontal boxsum = the two matrix sandwiches.

Precision: x and the constant matrices are cast to bf16 on the host. All
accumulations happen in fp32 PSUM; only element roundings are bf16, giving
~4e-3 relative error against the fp32 reference (the gate is 2e-2) while
halving HBM traffic (the kernel is HBM-bound) and doubling PE/DVE rates.

Layout: each core's shard is sent as [H, B_loc, C, W] (h-major) so every
SBUF partition receives one contiguous >=512B DRAM run per piece.

Schedule: every load is issued up-front, alternating between the two HWDGE
queues, so the HBM path streams without waiting on tile or semaphore
recycling (the previous revision's 8.6us of queue-idle gaps). Per batch,
pieces 0-1 fold on the tensor engine as bf16 identity-matmul quads into one
fp32 PSUM accumulator; pieces 2-3 fold on the vector engine as in-place
binary trees. The partial sums are never merged into one xs: uv accumulates
all three (quad partial + two tree results) with back-to-back matmuls
against [T | Bm.T]. Batch 0's uv consumes the four evac column groups
directly (one dependency hop after the ACT evac, so its whole sandwich
schedules ahead of batch 1's quads and the two chains never collide on
PE/ACT); batch 1 folds the groups on DVE first so the fewest matmuls
remain after the final piece lands on its tail-critical path. The
activation engine does every PSUM evacuation, stage 2 multiplies the uv
halves by A and T, bias adds straight out of PSUM in fp32, and both
stores ride the sync queue (a store issue on the scalar queue would block
the other batch's ACT ops behind it). Engine assignments were A/B'd on hardware; notable dead ends:
GpSimd folds (DVE's 2-port bf16 mode locks GpSimd out of SBUF, +6us), fp32r
matmuls (all-zero output on TRN2), single-queue streaming (~250 GB/s vs
~310 combined), and batch-role-aware queue layouts (cross-queue landing
jitter flips their intended orderings run to run).
"""

import numpy as np

B, C, H, W = 16, 32, 128, 128
N_CORES = 8
B_LOC = B // N_CORES

_PROGRAM = None
LAST_RESULTS = None


def _build_program():
    import concourse.mybir as mybir
    from concourse import bacc
    from concourse.bass import MemorySpace
    from concourse.tile import TileContext

    f32 = mybir.dt.float32
    bf16 = mybir.dt.bfloat16
    nc = bacc.Bacc(
        "TRN2", target_bir_lowering=False, debug=False, num_devices=N_CORES
    )

    x_dram = nc.dram_tensor("x", (H, B_LOC, C, W), bf16, kind="ExternalInput")
    # fused constants: [T | Bm.T | A | I | bias] as bf16 columns; the last
    # two columns hold each partition's fp32 bias value as raw bits
    cm_dram = nc.dram_tensor("cmat", (H, 4 * W + 2), bf16, kind="ExternalInput")
    out_dram = nc.dram_tensor("out", (H, B_LOC, W), f32, kind="ExternalOutput")

    x_ap = x_dram.ap()
    out_ap = out_dram.ap()

    PIECE = 8  # channels per full piece

    with TileContext(nc) as tc:
        with (
            tc.tile_pool(name="consts", bufs=1) as cpool,
            tc.tile_pool(name="xpool", bufs=2) as xpool,
            tc.tile_pool(name="spool", bufs=2) as spool,
            tc.tile_pool(name="psum", bufs=2, space=MemorySpace.PSUM) as ppool,
        ):
            cm_sb = cpool.tile([H, 4 * W + 2], bf16)
            t_sb = cm_sb[:, 0:W]
            tbm_sb = cm_sb[:, 0 : 2 * W]
            a_sb = cm_sb[:, 2 * W : 3 * W]
            i_sb = cm_sb[:, 3 * W : 4 * W]
            bias_sb = cm_sb[:, 4 * W : 4 * W + 2].bitcast(f32)

            # ---- phase 0: every load issued up-front. Pieces 0/1 are PE
            # quad pieces, pieces 2/3 DVE tree pieces; 8 channels each.
            tiles = {}
            for b in range(B_LOC):
                for p in range(4):
                    tiles[(b, p)] = xpool.tile(
                        [H, PIECE * W], bf16, tag=f"xq{p}", name=f"xq{b}{p}"
                    )

            def load(eng, b, p, c0, c1, col0):
                xq = tiles[(b, p)]
                eng.dma_start(
                    out=xq[:, col0 * W : (col0 + (c1 - c0)) * W].rearrange(
                        "h (c w) -> h c w", w=W
                    ),
                    in_=x_ap[:, b, c0:c1, :],
                )

            nc.scalar.dma_start(out=cm_sb, in_=cm_dram.ap())
            order = [(0, 0), (0, 1), (0, 2), (0, 3), (1, 0), (1, 1), (1, 2)]
            for idx, (b, p) in enumerate(order):
                eng = nc.sync if idx % 2 == 0 else nc.scalar
                load(eng, b, p, p * PIECE, (p + 1) * PIECE, 0)
            # batch 1's final piece arrives as a 7ch run plus a 1ch sliver
            # landing last, so the tail fold is a single merge add instead
            # of a 3-op tree
            load(nc.scalar, 1, 3, 24, 31, 0)
            load(nc.sync, 1, 3, 31, 32, 7)

            # ---- phase 1, per batch
            for b in range(B_LOC):
                # PE: 4 identity quads fold pieces 0-1 into fp32 PSUM
                quad_pieces = (0, 1)
                nquads = 2 * len(quad_pieces)
                cs_psum = ppool.tile([H, 4 * W], f32, tag="csp")
                q = 0
                for p in quad_pieces:
                    for half in range(2):
                        nc.tensor.matmul(
                            cs_psum,
                            i_sb,
                            tiles[(b, p)][:, half * 4 * W : (half + 1) * 4 * W],
                            start=(q == 0),
                            stop=(q == nquads - 1),
                        )
                        q += 1
                # ACT evacuates the 4-way partial (rounding to bf16)
                cs = spool.tile([H, 4 * W], bf16, tag="cs")
                nc.scalar.copy(cs, cs_psum)

                # DVE folds the quad partial (batch 1 only; batch 0's uv
                # consumes the evac groups directly)
                if b != 0:
                    nc.vector.tensor_add(
                        cs[:, 0 : 2 * W], cs[:, 0 : 2 * W], cs[:, 2 * W : 4 * W]
                    )
                    nc.vector.tensor_add(cs[:, 0:W], cs[:, 0:W], cs[:, W : 2 * W])
                tree_pieces = (2, 3)
                xq = tiles[(b, 2)]
                n = PIECE * W
                while n > W:
                    n //= 2
                    nc.vector.tensor_add(xq[:, :n], xq[:, :n], xq[:, n : 2 * n])
                p3 = tiles[(b, 3)]
                if b == 0:
                    n = PIECE * W
                    while n > W:
                        n //= 2
                        nc.vector.tensor_add(p3[:, :n], p3[:, :n], p3[:, n : 2 * n])
                else:
                    nc.vector.tensor_add(
                        p3[:, 0 : 2 * W], p3[:, 0 : 2 * W], p3[:, 2 * W : 4 * W]
                    )
                    nc.vector.tensor_add(
                        p3[:, 0 : 2 * W], p3[:, 0 : 2 * W], p3[:, 4 * W : 6 * W]
                    )
                    nc.vector.tensor_add(p3[:, 0:W], p3[:, 0:W], p3[:, W : 2 * W])
                    nc.vector.tensor_add(p3[:, 0:W], p3[:, 0:W], p3[:, 6 * W : 7 * W])
                    nc.vector.tensor_add(p3[:, 0:W], p3[:, 0:W], p3[:, 7 * W : 8 * W])

                # PE: uv accumulates every partial sum against [T | Bm.T]
                uv_psum = ppool.tile([H, 2 * W], f32, tag="uvp")
                cs_parts = (
                    [cs[:, g * W : (g + 1) * W] for g in range(4)]
                    if b == 0
                    else [cs[:, 0:W]]
                )
                for i, part in enumerate(cs_parts):
                    nc.tensor.matmul(
                        uv_psum, part, tbm_sb, start=(i == 0), stop=False
                    )
                for p in tree_pieces:
                    nc.tensor.matmul(
                        uv_psum,
                        tiles[(b, p)][:, 0:W],
                        tbm_sb,
                        start=False,
                        stop=(p == 3),
                    )
                uv = spool.tile([H, 2 * W], bf16, tag="uv")
                # batch 1's evacuation is split ACT || DVE so stage-2's
                # second matmul is not queued behind a serial ACT copy
                nc.scalar.copy(uv[:, 0:W], uv_psum[:, 0:W])
                if b == 0:
                    nc.scalar.copy(uv[:, W : 2 * W], uv_psum[:, W : 2 * W])
                else:
                    nc.vector.tensor_copy(uv[:, W : 2 * W], uv_psum[:, W : 2 * W])

                op = ppool.tile([H, W], f32, tag="op")
                nc.tensor.matmul(op, uv[:, 0:W], a_sb, start=True, stop=False)
                nc.tensor.matmul(op, uv[:, W : 2 * W], t_sb, start=False, stop=True)

                o2b = spool.tile([H, W], f32, tag="o2")
                nc.scalar.add(o2b, op, add=bias_sb)
                nc.sync.dma_start(out=out_ap[:, b, :], in_=o2b)

    nc.compile()
    return nc


def _get_program():
    global _PROGRAM
    if _PROGRAM is None:
        _PROGRAM = _build_program()
    return _PROGRAM


def _build_consts(K, bias, lambda_c, lambda_a, np_bf16):
    K = np.asarray(K, np.float32)
    lc = float(np.asarray(lambda_c))
    la = float(np.asarray(lambda_a))
    a = (lc + la) * K.sum(axis=0)  # column sums -> horizontal taps
    b = (la - lc) * K.sum(axis=1)  # row sums -> vertical taps
    eye = np.eye(H, dtype=np.float32)
    up = np.eye(H, k=1, dtype=np.float32)
    dn = np.eye(H, k=-1, dtype=np.float32)
    T = eye + up + dn
    A = a[1] * eye + a[0] * up + a[2] * dn
    Bm = b[1] * eye + b[2] * up + b[0] * dn
    # fused [T | Bm.T | A | I] in bf16, then the fp32 bias bit-packed into
    # two trailing bf16 columns
    cm = np.concatenate([T, Bm.T, A, eye], axis=1).astype(np_bf16)
    bias_col = np.full(
        (H, 1), np.asarray(bias, np.float32).reshape(-1)[0], np.float32
    )
    bias_bits = bias_col.view(np.uint16).view(np_bf16)  # [H, 2] raw halves
    return np.ascontiguousarray(np.concatenate([cm, bias_bits], axis=1))


def kernel(x, K, bias, lambda_c, lambda_a, _trace=False):
    global LAST_RESULTS
    import concourse.mybir as mybir
    from concourse.bass_utils import run_bass_kernel_spmd

    np_bf16 = mybir.dt.np(mybir.dt.bfloat16)
    x = np.asarray(x, np.float32)
    cm = _build_consts(K, bias, lambda_c, lambda_a, np_bf16)
    nc = _get_program()

    in_maps = []
    for core in range(N_CORES):
        shard = x[core * B_LOC : (core + 1) * B_LOC]  # [B_LOC, C, H, W]
        shard_t = np.ascontiguousarray(
            shard.transpose(2, 0, 1, 3).astype(np_bf16)
        )  # [H, B, C, W] bf16
        in_maps.append({"x": shard_t, "cmat": cm})

    res = run_bass_kernel_spmd(
        nc, in_maps, core_ids=list(range(N_CORES)), trace=_trace
    )
    LAST_RESULTS = res
    # per-core outputs are [H, B_LOC, W]; swap back to [B_LOC, H, W]
    out = np.concatenate(
        [r["out"].transpose(1, 0, 2) for r in res.results], axis=0
    )
    return out.reshape(B, 1, H, W).astype(np.float32, copy=False)
